# revision 4
# baseline (speedup 1.0000x reference)
"""Trainium2 Bass kernel for nn_DilatedContextAttentionModule.

Reference computation (per batch element b, C=256, N=64*64=4096):
    g    = G xj + g_b 1^T
    th   = T xi + t_b 1^T
    phi  = P xj + p_b 1^T
    f    = th^T phi / N          (N x N, linear -- no softmax)
    y    = reshape(f @ g^T)      y[c,n] = sum_m f[n,m] g[c,m]
    z    = W y + W_b 1^T + xi
    out  = BatchNorm2d(z)        (training stats over the whole batch)

Because f is linear, matrix associativity collapses the two N x N
matmuls into C x C ones:
    y = (1/N) (g phi^T) th = (1/N) S th
    z = E' xi + d 1^T + xi
    E' = (1/N) W S T,   d = (1/N) W S t_b + W_b
    S  = g0 phi0^T + (G sxj + N g_b) p_b^T + g_b (P sxj)^T
         where g0 = G xj, phi0 = P xj, sxj = xj @ 1.

This drops ~9.7 GMACs/batch to ~0.9 GMACs/batch.  Sharding is
data-parallel over batch (1 element per core, 8 cores); the BN
batch stats (per-channel mean / mean-of-square) are combined with a
2 KB cross-core AllReduce.
"""

import numpy as np

import concourse.bass as bass
import concourse.bacc as bacc
import concourse.tile as tile
from concourse import mybir
from concourse import bass_utils

B = 8
C = 256
N = 4096          # 64 * 64
NCORES = 8
NCH = 2           # channel chunks of 128
NT = 32           # n chunks of 128 (phase 1)
NZ = 8            # n tiles of 512 (phase 3)
F32 = mybir.dt.float32
BN_EPS = 1e-5

# TensorE compute dtype for the big matmuls. float32r streams at
# 1 cycle/row (vs 4 for float32) when the moving free dim >= 256, but
# requires all producers to round their outputs to float32r.
import os as _os
MM_DT = {
    "f32": mybir.dt.float32,
    "f32r": mybir.dt.float32r,
    "bf16": mybir.dt.bfloat16,
}[_os.environ.get("MM_DT", "f32")]


def _mm(x: bass.AP) -> bass.AP:
    if MM_DT == F32:
        return x
    return x.bitcast(MM_DT)


def build_kernel(nc) -> None:
    f32 = F32
    xi_d = nc.dram_tensor("xi", [C, N], f32, kind="ExternalInput").ap()
    xj_d = nc.dram_tensor("xj", [C, N], f32, kind="ExternalInput").ap()
    # [128, 2, 512]: packed per-chunk conv weights [G^T | P^T]
    wgp_d = nc.dram_tensor("wgp", [128, NCH, 512], f32, kind="ExternalInput").ap()
    # [128, 2, 256]: theta_w rows (lhsT for E'^T), chunked on cp
    wtw_d = nc.dram_tensor("wtw", [128, NCH, C], f32, kind="ExternalInput").ap()
    # [128, 2, 256]: (W_w^T / N) rows, chunked on cg
    wwt_d = nc.dram_tensor("wwt", [128, NCH, C], f32, kind="ExternalInput").ap()
    # [128, 2]: theta_b column, chunked
    wtb_d = nc.dram_tensor("wtb", [128, NCH], f32, kind="ExternalInput").ap()
    # [1, 1024]: rows [N*g_b | g_b | p_b | W_b]
    aux_d = nc.dram_tensor("aux", [1, 4 * C], f32, kind="ExternalInput").ap()
    # [128, 2, 2]: (gamma, beta) per channel, chunked
    gbe_d = nc.dram_tensor("gbe", [128, NCH, 2], f32, kind="ExternalInput").ap()
    out_d = nc.dram_tensor("out", [C, N], f32, kind="ExternalOutput").ap()

    with tile.TileContext(nc) as tc:
        _body(tc, xi_d, xj_d, wgp_d, wtw_d, wwt_d, wtb_d, aux_d, gbe_d, out_d)


def _body(tc, xi_d, xj_d, wgp_d, wtw_d, wwt_d, wtb_d, aux_d, gbe_d, out_d):
    nc = tc.nc
    f32 = F32
    import contextlib

    with contextlib.ExitStack() as ctx:
        constp = ctx.enter_context(tc.tile_pool(name="const", bufs=1))
        datap = ctx.enter_context(tc.tile_pool(name="data", bufs=1))
        workp = ctx.enter_context(tc.tile_pool(name="work", bufs=4))
        rowsp = ctx.enter_context(tc.tile_pool(name="rows", bufs=2))
        psbig = ctx.enter_context(tc.tile_pool(name="ps_big", bufs=3, space="PSUM"))
        psacc = ctx.enter_context(tc.tile_pool(name="ps_acc", bufs=2, space="PSUM"))
        pssml = ctx.enter_context(tc.tile_pool(name="ps_sml", bufs=1, space="PSUM"))
        dramp = ctx.enter_context(tc.tile_pool(name="dram", bufs=2, space="DRAM"))

        # ---- constants / weights ------------------------------------
        w_gp = constp.tile([128, NCH, 512], f32, tag="w_gp")
        nc.sync.dma_start(out=w_gp, in_=wgp_d)
        w_tw = constp.tile([128, NCH, C], f32, tag="w_tw")
        nc.sync.dma_start(out=w_tw, in_=wtw_d)
        w_wt = constp.tile([128, NCH, C], f32, tag="w_wt")
        nc.sync.dma_start(out=w_wt, in_=wwt_d)
        w_tb = constp.tile([128, NCH], f32, tag="w_tb")
        nc.sync.dma_start(out=w_tb, in_=wtb_d)
        aux = constp.tile([1, 4 * C], f32, tag="aux")
        nc.sync.dma_start(out=aux, in_=aux_d)
        gbe = constp.tile([128, NCH, 2], f32, tag="gbe")
        nc.sync.dma_start(out=gbe, in_=gbe_d)
        ones = constp.tile([1, 512], f32, tag="ones")
        nc.vector.memset(ones, 1.0)
        eps = constp.tile([128, 1], f32, tag="eps")
        nc.vector.memset(eps, BN_EPS)

        # ---- big data tiles -----------------------------------------
        xj_s = []
        for k in range(NCH):
            t = datap.tile([128, N], f32, tag=f"xj{k}")
            nc.sync.dma_start(out=t, in_=xj_d[k * 128:(k + 1) * 128, :])
            xj_s.append(t)
        xi_s = []
        for k in range(NCH):
            t = datap.tile([128, N], f32, tag=f"xi{k}")
            nc.sync.dma_start(out=t, in_=xi_d[k * 128:(k + 1) * 128, :])
            xi_s.append(t)

        # ---- sxj = rowsum(xj); bias-correction rows ------------------
        sxj = rowsp.tile([128, NCH], f32, tag="sxj")
        for k in range(NCH):
            nc.vector.reduce_sum(
                out=sxj[:, k:k + 1], in_=xj_s[k], axis=mybir.AxisListType.X
            )
        # s_g0_row = sxj^T @ G^T, s_phi0_row = sxj^T @ P^T   (each [1, 256])
        srow_ps = pssml.tile([1, 2 * C], f32, tag="sml")
        for k in range(NCH):
            nc.tensor.matmul(
                srow_ps[:, 0:C],
                _mm(sxj[:, k:k + 1]),
                _mm(w_gp[:, k, 0:C]),
                start=(k == 0), stop=(k == NCH - 1),
            )
        for k in range(NCH):
            nc.tensor.matmul(
                srow_ps[:, C:2 * C],
                _mm(sxj[:, k:k + 1]),
                _mm(w_gp[:, k, C:2 * C]),
                start=(k == 0), stop=(k == NCH - 1),
            )
        # u_row = s_g0 + N*g_b ; v_row = s_phi0
        urow = rowsp.tile([1, C], f32, tag="urow")
        nc.vector.tensor_add(urow, srow_ps[:, 0:C], aux[:, 0:C])
        vrow = rowsp.tile([1, C], f32, tag="vrow")
        nc.vector.tensor_copy(vrow, srow_ps[:, C:2 * C])

        # ---- phase 1: S = g0 phi0^T (+ rank-1 bias corrections) -----
        S_ps = [psacc.tile([128, C], f32, tag="acc", name=f"S_ps{m}") for m in range(NCH)]
        for i in range(NT):
            sl = slice(i * 128, (i + 1) * 128)
            gp_ps = psbig.tile([128, 512], f32, tag="big")
            for k in range(NCH):
                nc.tensor.matmul(
                    gp_ps, _mm(xj_s[k][:, sl]), _mm(w_gp[:, k, :]),
                    start=(k == 0), stop=(k == NCH - 1),
                )
            gpt = workp.tile([128, 512], f32, tag="gpt")
            nc.vector.tensor_copy(gpt, gp_ps)
            for m in range(NCH):
                nc.tensor.matmul(
                    S_ps[m],
                    _mm(gpt[:, m * 128:(m + 1) * 128]),
                    _mm(gpt[:, C:2 * C]),
                    start=(i == 0), stop=False,
                )
        for m in range(NCH):
            msl = slice(m * 128, (m + 1) * 128)
            nc.tensor.matmul(
                S_ps[m], _mm(urow[:, msl]), _mm(aux[:, 2 * C:3 * C]),
                start=False, stop=False,
            )
            nc.tensor.matmul(
                S_ps[m], _mm(aux[:, C + m * 128:C + (m + 1) * 128]), _mm(vrow),
                start=False, stop=True,
            )
        S_sb = []
        for m in range(NCH):
            t = workp.tile([128, C], f32, tag=f"S{m}")
            nc.scalar.copy(t, S_ps[m])
            S_sb.append(t)

        # ---- phase 2: V = S^T (W^T/N);  E'^T = T^T V;  d = V^T t_b --
        V_sb = []
        for m in range(NCH):
            v_ps = psacc.tile([128, C], f32, tag="acc")
            msl = slice(m * 128, (m + 1) * 128)
            for k in range(NCH):
                nc.tensor.matmul(
                    v_ps, _mm(S_sb[k][:, msl]), _mm(w_wt[:, k, :]),
                    start=(k == 0), stop=(k == NCH - 1),
                )
            t = workp.tile([128, C], f32, tag=f"V{m}")
            nc.scalar.copy(t, v_ps)
            V_sb.append(t)
        ET_sb = []
        for m in range(NCH):
            e_ps = psacc.tile([128, C], f32, tag="acc")
            msl = slice(m * 128, (m + 1) * 128)
            for k in range(NCH):
                nc.tensor.matmul(
                    e_ps, _mm(w_tw[:, k, msl]), _mm(V_sb[k]),
                    start=(k == 0), stop=(k == NCH - 1),
                )
            t = workp.tile([128, C], f32, tag=f"ET{m}")
            nc.scalar.copy(t, e_ps)
            ET_sb.append(t)
        d_ps = pssml.tile([1, C], f32, tag="sml")
        for k in range(NCH):
            nc.tensor.matmul(
                d_ps, _mm(w_tb[:, k:k + 1]), _mm(V_sb[k]),
                start=(k == 0), stop=(k == NCH - 1),
            )
        drow = rowsp.tile([1, C], f32, tag="drow_sb")
        nc.vector.tensor_add(drow, d_ps, aux[:, 3 * C:4 * C])

        # ---- phase 3: z = E'^T.T @ xi + d 1^T + xi ------------------
        z_s = []
        for j in range(NCH):
            t = datap.tile([128, N], f32, tag=f"z{j}")
            z_s.append(t)
        for j in range(NCH):
            jsl = slice(j * 128, (j + 1) * 128)
            for tix in range(NZ):
                tsl = slice(tix * 512, (tix + 1) * 512)
                z_ps = psbig.tile([128, 512], f32, tag="big")
                for k in range(NCH):
                    nc.tensor.matmul(
                        z_ps, _mm(ET_sb[k][:, jsl]), _mm(xi_s[k][:, tsl]),
                        start=(k == 0), stop=False,
                    )
                nc.tensor.matmul(
                    z_ps, _mm(drow[:, jsl]), _mm(ones),
                    start=False, stop=True,
                )
                nc.vector.tensor_add(z_s[j][:, tsl], z_ps, xi_s[j][:, tsl])

        # ---- BN stats: per-channel mean / mean(x^2), cross-core sum -
        spack = rowsp.tile([128, 4], f32, tag="spack")
        for j in range(NCH):
            z3 = z_s[j].rearrange("p (s f) -> p s f", f=512)
            stats = workp.tile([128, NZ, 6], f32, tag="bnst")
            for s in range(NZ):
                nc.vector.bn_stats(out=stats[:, s, :], in_=z3[:, s, :])
            mv = rowsp.tile([128, 2], f32, tag="mv")
            nc.vector.bn_aggr(out=mv, in_=stats)
            nc.vector.tensor_copy(spack[:, j:j + 1], mv[:, 0:1])
            # mean^2 + var  (= mean of squares)
            nc.vector.scalar_tensor_tensor(
                out=spack[:, 2 + j:3 + j], in0=mv[:, 0:1], scalar=mv[:, 0:1],
                in1=mv[:, 1:2],
                op0=mybir.AluOpType.mult, op1=mybir.AluOpType.add,
            )
        cc_in = dramp.tile([128, 4], f32, tag="cc_in")
        cc_out = dramp.tile([128, 4], f32, tag="cc_out")
        nc.sync.dma_start(out=cc_in, in_=spack)
        nc.gpsimd.collective_compute(
            "AllReduce",
            mybir.AluOpType.add,
            replica_groups=[list(range(NCORES))],
            ins=[cc_in.opt()],
            outs=[cc_out.opt()],
        )
        ssum = rowsp.tile([128, 4], f32, tag="ssum")
        nc.sync.dma_start(out=ssum, in_=cc_out)

        # ---- normalize + affine + store -----------------------------
        for j in range(NCH):
            mcol = rowsp.tile([128, 1], f32, tag="mcol")
            nc.vector.tensor_scalar_mul(mcol, ssum[:, j:j + 1], 1.0 / NCORES)
            qcol = rowsp.tile([128, 1], f32, tag="qcol")
            nc.vector.tensor_scalar_mul(qcol, ssum[:, 2 + j:3 + j], 1.0 / NCORES)
            nmcol = rowsp.tile([128, 1], f32, tag="nmcol")
            nc.vector.tensor_scalar_mul(nmcol, mcol, -1.0)
            # var = q - m^2 = (m * -m) + q
            vcol = rowsp.tile([128, 1], f32, tag="vcol")
            nc.vector.scalar_tensor_tensor(
                out=vcol, in0=mcol, scalar=nmcol, in1=qcol,
                op0=mybir.AluOpType.mult, op1=mybir.AluOpType.add,
            )
            # rstd = 1 / sqrt(var + eps)
            scol = rowsp.tile([128, 1], f32, tag="scol")
            nc.scalar.activation(
                out=scol, in_=vcol, func=mybir.ActivationFunctionType.Sqrt,
                bias=eps, scale=1.0,
            )
            nc.vector.reciprocal(out=scol, in_=scol)
            acol = rowsp.tile([128, 1], f32, tag="acol")
            nc.vector.tensor_mul(acol, scol, gbe[:, j, 0:1])
            nacol = rowsp.tile([128, 1], f32, tag="nacol")
            nc.vector.tensor_scalar_mul(nacol, acol, -1.0)
            bcol = rowsp.tile([128, 1], f32, tag="bcol")
            nc.vector.scalar_tensor_tensor(
                out=bcol, in0=mcol, scalar=nacol, in1=gbe[:, j, 1:2],
                op0=mybir.AluOpType.mult, op1=mybir.AluOpType.add,
            )
            nc.vector.tensor_scalar(
                out=z_s[j], in0=z_s[j], scalar1=acol, scalar2=bcol,
                op0=mybir.AluOpType.mult, op1=mybir.AluOpType.add,
            )
            nc.sync.dma_start(out=out_d[j * 128:(j + 1) * 128, :], in_=z_s[j])


_NC_CACHE: dict = {}


def _get_nc():
    if "nc" not in _NC_CACHE:
        nc = bacc.Bacc(
            "TRN2",
            target_bir_lowering=False,
            debug=False,
            enable_asserts=True,
            num_devices=NCORES,
        )
        build_kernel(nc)
        nc.compile()
        _NC_CACHE["nc"] = nc
    return _NC_CACHE["nc"]


def _make_in_maps(inputs: dict) -> list[dict]:
    xi = np.ascontiguousarray(np.asarray(inputs["xi"], np.float32).reshape(B, C, N))
    xj = np.ascontiguousarray(np.asarray(inputs["xj"], np.float32).reshape(B, C, N))
    g_w = np.asarray(inputs["g_w"], np.float32)
    g_b = np.asarray(inputs["g_b"], np.float32)
    t_w = np.asarray(inputs["theta_w"], np.float32)
    t_b = np.asarray(inputs["theta_b"], np.float32)
    p_w = np.asarray(inputs["phi_w"], np.float32)
    p_b = np.asarray(inputs["phi_b"], np.float32)
    W_w = np.asarray(inputs["W_w"], np.float32)
    W_b = np.asarray(inputs["W_b"], np.float32)
    gam = np.asarray(inputs["bn_gamma"], np.float32)
    bet = np.asarray(inputs["bn_beta"], np.float32)

    def chunked(a):  # [256, F] -> [128, 2, F]
        return np.ascontiguousarray(a.reshape(2, 128, -1).transpose(1, 0, 2))

    wgp = chunked(np.concatenate([g_w.T, p_w.T], axis=1))          # [128,2,512]
    wtw = chunked(t_w)                                             # [128,2,256]
    wwt = chunked(W_w.T * (1.0 / N))                               # [128,2,256]
    wtb = np.ascontiguousarray(t_b.reshape(2, 128).T)              # [128,2]
    aux = np.concatenate([N * g_b, g_b, p_b, W_b])[None, :]        # [1,1024]
    aux = np.ascontiguousarray(aux.astype(np.float32))
    gbe = chunked(np.stack([gam, bet], axis=1))                    # [128,2,2]

    in_maps = []
    for b in range(B):
        in_maps.append({
            "xi": xi[b], "xj": xj[b],
            "wgp": wgp, "wtw": wtw, "wwt": wwt, "wtb": wtb,
            "aux": aux, "gbe": gbe,
        })
    return in_maps


def kernel(**inputs) -> np.ndarray:
    nc = _get_nc()
    in_maps = _make_in_maps(inputs)
    res = bass_utils.run_bass_kernel_spmd(
        nc, in_maps, core_ids=list(range(NCORES)),
    )
    out = np.stack([res.results[c]["out"] for c in range(NCORES)])
    return np.ascontiguousarray(out.reshape(B, C, 64, 64).astype(np.float32))


if __name__ == "__main__":
    rng = np.random.default_rng(0)
    fake = {
        "xi": rng.standard_normal((B, C, 64, 64), np.float32),
        "xj": rng.standard_normal((B, C, 64, 64), np.float32),
        "g_w": rng.standard_normal((C, C), np.float32) / 16,
        "g_b": rng.standard_normal((C,), np.float32) / 16,
        "theta_w": rng.standard_normal((C, C), np.float32) / 16,
        "theta_b": rng.standard_normal((C,), np.float32) / 16,
        "phi_w": rng.standard_normal((C, C), np.float32) / 16,
        "phi_b": rng.standard_normal((C,), np.float32) / 16,
        "W_w": rng.standard_normal((C, C), np.float32) / 16,
        "W_b": rng.standard_normal((C,), np.float32) / 16,
        "bn_gamma": np.ones((C,), np.float32),
        "bn_beta": np.zeros((C,), np.float32),
    }
    out = kernel(**fake)
    print("out", out.shape, out.dtype, float(np.abs(out).mean()))
